# revision 15
# baseline (speedup 1.0000x reference)
"""Trainium2 Bass kernel: ResNet-18 backbone + conditional MoE heads.

Data-parallel across 8 NeuronCores, 8 images per core. The full backbone and
heads run on-device; host work is data staging only (BN folding, weight
transposes, conv1 space-to-depth+im2col, one-hot command masks).

Per-core device plan (index math validated against the jax reference by a
numpy mirror):
  conv1   : image PAIRS packed K=96 (2x48 s2d+im2col-x rows), M=128 (2x64),
            4 accumulating matmuls per output chunk; input streamed in
            row-chunk tiles [96,19,112]
  maxpool : two separable pool_max passes (chunked horizontal pass)
  stage1  : pairs via block-diag weights, K=128, M=128; 9 shifted matmuls/conv
  stage2  : per image (K=64 -> M=128)
  stage3  : per pair, 2 images batched in the matmul free dim
  stage4  : all 8 images batched in the matmul free dim, weights streamed
  heads   : one-hot-masked GEMMs; avgpool /49 folded into head W1
"""
import numpy as np

N_CORES = 8
B = 8  # images per core

_CACHE = {}


# ---------------------------------------------------------------------------
# host-side prep (numpy only)
# ---------------------------------------------------------------------------

def _fold_bn(w, bn):
    scale = np.asarray(bn['g'], np.float64) / np.sqrt(np.asarray(bn['v'], np.float64) + 1e-5)
    wf = np.asarray(w, np.float64) * scale[:, None, None, None]
    bias = np.asarray(bn['b'], np.float64) - np.asarray(bn['m'], np.float64) * scale
    return wf.astype(np.float32), bias.astype(np.float32)


def _w3x3_pos(w):
    return np.ascontiguousarray(w.transpose(2, 3, 1, 0).reshape(9, w.shape[1], w.shape[0]))


def _blockdiag2(w9):
    p, cin, cout = w9.shape
    out = np.zeros((p, 2 * cin, 2 * cout), np.float32)
    out[:, :cin, :cout] = w9
    out[:, cin:, cout:] = w9
    return out


def _conv1_transform(img):
    """[3,224,224] -> [48,115,112] (space-to-depth 2x2 + im2col over 4 x-taps)."""
    xpad = np.zeros((3, 230, 230), np.float32)
    xpad[:, 3:227, 3:227] = img
    s2d = xpad.reshape(3, 115, 2, 115, 2).transpose(0, 2, 4, 1, 3)
    c1 = np.empty((3, 2, 2, 4, 115, 112), np.float32)
    for b in range(4):
        c1[:, :, :, b, :, :] = s2d[:, :, :, :, b:b + 112]
    return c1.reshape(48, 115, 112)


def _conv1_weights(w64):
    wt = np.zeros((4, 3, 2, 2, 4, 64), np.float32)
    for a in range(4):
        for p in range(2):
            for b in range(4):
                for q in range(2):
                    ky, kx = 2 * a + p, 2 * b + q
                    if ky <= 6 and kx <= 6:
                        wt[a, :, p, q, b, :] = w64[:, :, ky, kx].T
    return wt.reshape(4, 48, 64)


def _prep_shared(params):
    bb = params['backbone']
    out = {}

    w1f, b1f = _fold_bn(bb['conv1'], bb['bn1'])
    wt = _conv1_weights(w1f)                                    # [4,48,64]
    c1wp = np.zeros((96, 4, 128), np.float32)                   # paired block-diag
    c1wp[0:48, :, 0:64] = wt.transpose(1, 0, 2)
    c1wp[48:96, :, 64:128] = wt.transpose(1, 0, 2)
    out['c1w'] = c1wp
    out['c1b'] = np.ascontiguousarray(np.tile(b1f, 2).reshape(128, 1))

    def blockw(blk):
        w1, b1 = _fold_bn(blk['conv1'], blk['bn1'])
        w2, b2 = _fold_bn(blk['conv2'], blk['bn2'])
        r = {'w1': _w3x3_pos(w1), 'b1': b1, 'w2': _w3x3_pos(w2), 'b2': b2}
        if 'down_conv' in blk:
            wd, bd = _fold_bn(blk['down_conv'], blk['down_bn'])
            r['wd'] = np.ascontiguousarray(wd[:, :, 0, 0].T)
            r['bd'] = bd
        return r

    st = [[blockw(b) for b in stage] for stage in bb['stages']]

    s1w = np.stack([_blockdiag2(w).transpose(1, 0, 2) for w in
                    (st[0][0]['w1'], st[0][0]['w2'], st[0][1]['w1'], st[0][1]['w2'])])
    out['s1w'] = np.ascontiguousarray(s1w.transpose(1, 0, 2, 3))        # [128K,4conv,9,128M]
    out['s1b'] = np.ascontiguousarray(np.stack(
        [np.tile(b, 2) for b in (st[0][0]['b1'], st[0][0]['b2'],
                                 st[0][1]['b1'], st[0][1]['b2'])], axis=1))  # [128,4]

    _w21 = st[1][0]['w1'].transpose(1, 0, 2)  # [64K,9,128]
    out['s2w_c1'] = np.ascontiguousarray(np.concatenate([_w21, _w21], axis=0))  # [128,9,128]
    out['s2w_r'] = np.ascontiguousarray(np.stack(
        [w.transpose(1, 0, 2) for w in
         (st[1][0]['w2'], st[1][1]['w1'], st[1][1]['w2'])]).transpose(1, 0, 2, 3))  # [128,3,9,128]
    out['s2wd'] = np.ascontiguousarray(np.concatenate([st[1][0]['wd']] * 2, axis=0))  # [128,128]
    out['s2b'] = np.ascontiguousarray(np.stack(
        [st[1][0]['b1'], st[1][0]['b2'], st[1][1]['b1'], st[1][1]['b2']], axis=1))
    out['s2bd'] = np.ascontiguousarray(st[1][0]['bd'].reshape(128, 1))

    def split_w(w9, cbi_n, cbo_n):
        p = w9.shape[0]
        return w9.reshape(p, cbi_n, 128, cbo_n, 128).transpose(1, 2, 0, 3, 4)  # [cbi,K,9,cbo,M]

    out['s3w_c1'] = np.ascontiguousarray(split_w(st[2][0]['w1'], 1, 2)[0])    # [128,9,2,128]
    out['s3w_r'] = np.ascontiguousarray(np.stack(
        [split_w(w, 2, 2) for w in (st[2][0]['w2'], st[2][1]['w1'], st[2][1]['w2'])]
    ).transpose(0, 2, 1, 3, 4, 5))                                            # [3,128,2cbi,9,2,128]
    out['s3wd'] = np.ascontiguousarray(st[2][0]['wd'].reshape(128, 2, 128))
    out['s3b'] = np.ascontiguousarray(np.stack(
        [b.reshape(2, 128).T for b in
         (st[2][0]['b1'], st[2][0]['b2'], st[2][1]['b1'], st[2][1]['b2'])], axis=1))  # [128,4,2]
    out['s3bd'] = np.ascontiguousarray(st[2][0]['bd'].reshape(2, 128).T)

    out['s4w_c1'] = np.ascontiguousarray(split_w(st[3][0]['w1'], 2, 4))       # [2,128,9,4,128]
    out['s4w_r'] = np.ascontiguousarray(np.stack(
        [split_w(w, 4, 4) for w in (st[3][0]['w2'], st[3][1]['w1'], st[3][1]['w2'])]))  # [3,4,128,9,4,128]
    out['s4wd'] = np.ascontiguousarray(
        st[3][0]['wd'].reshape(2, 128, 4, 128).transpose(1, 0, 2, 3))         # [128,2,4,128]
    out['s4b'] = np.ascontiguousarray(np.stack(
        [b.reshape(4, 128).T for b in
         (st[3][0]['b1'], st[3][0]['b2'], st[3][1]['b1'], st[3][1]['b2'])], axis=1))  # [128,4,4]
    out['s4bd'] = np.ascontiguousarray(st[3][0]['bd'].reshape(4, 128).T)

    for hn in ('lane', 'route', 'tld', 'tls'):
        hp = params[hn]
        w1 = np.asarray(hp['W1'], np.float32) / 49.0
        b1 = np.asarray(hp['b1'], np.float32)
        w2 = np.asarray(hp['W2'], np.float32)
        b2 = np.asarray(hp['b2'], np.float32)
        if w1.ndim == 3:
            out[f'{hn}_w1'] = np.ascontiguousarray(
                w1.reshape(4, 4, 128, 64).reshape(16, 128, 64).transpose(1, 0, 2))  # [128,16,64]
            out[f'{hn}_b1'] = np.ascontiguousarray(b1)                 # [4,64]
        else:
            out[f'{hn}_w1'] = np.ascontiguousarray(
                w1.reshape(4, 128, 64).transpose(1, 0, 2))             # [128,4,64]
            out[f'{hn}_b1'] = np.ascontiguousarray(b1.reshape(1, 64))
        out[f'{hn}_w2'] = np.ascontiguousarray(w2)
        out[f'{hn}_b2'] = np.ascontiguousarray(b2.reshape(1, -1))
    return out


def _prep_core(img8, cmd8):
    c1in = np.stack([_conv1_transform(np.asarray(img8[i], np.float32)) for i in range(B)])
    cmd = np.asarray(cmd8).astype(np.int64)
    oh = np.zeros((4, 8), np.float32)
    oh[cmd, np.arange(8)] = 1.0
    maskrep = np.ascontiguousarray(
        np.broadcast_to(oh[None, :, None, :], (128, 4, 4, 8)))  # [p, e, cb, img]
    return {'c1in': c1in, 'oh': np.ascontiguousarray(oh), 'maskrep': maskrep}


# ---------------------------------------------------------------------------
# bass program
# ---------------------------------------------------------------------------

def _build_bass():
    import concourse.bass as bass
    import concourse.mybir as mybir
    import concourse.tile as tile
    from concourse import bacc
    from contextlib import ExitStack

    f32 = mybir.dt.float32
    AF = mybir.ActivationFunctionType
    AX = mybir.AxisListType

    nc = bacc.Bacc(None, target_bir_lowering=False)

    def din(name, shape):
        return nc.declare_dram_parameter(name, list(shape), f32, isOutput=False)

    c1in_d = din('c1in', (8, 48, 115, 112))
    c1w_d = din('c1w', (96, 4, 128))
    c1b_d = din('c1b', (128, 1))
    s1w_d = din('s1w', (128, 4, 9, 128))
    s1b_d = din('s1b', (128, 4))
    s2wc1_d = din('s2w_c1', (128, 9, 128))
    s2wr_d = din('s2w_r', (128, 3, 9, 128))
    s2wd_d = din('s2wd', (128, 128))
    s2b_d = din('s2b', (128, 4))
    s2bd_d = din('s2bd', (128, 1))
    s3wc1_d = din('s3w_c1', (128, 9, 2, 128))
    s3wr_d = din('s3w_r', (3, 128, 2, 9, 2, 128))
    s3wd_d = din('s3wd', (128, 2, 128))
    s3b_d = din('s3b', (128, 4, 2))
    s3bd_d = din('s3bd', (128, 2))
    s4wc1_d = din('s4w_c1', (2, 128, 9, 4, 128))
    s4wr_d = din('s4w_r', (3, 4, 128, 9, 4, 128))
    s4wd_d = din('s4wd', (128, 2, 4, 128))
    s4b_d = din('s4b', (128, 4, 4))
    s4bd_d = din('s4bd', (128, 4))
    hd = {}
    for hn, odim in (('lane', 1), ('route', 1), ('tld', 1), ('tls', 2)):
        cond = hn in ('lane', 'route')
        hd[hn] = {
            'w1': din(f'{hn}_w1', (128, 16, 64) if cond else (128, 4, 64)),
            'b1': din(f'{hn}_b1', (4, 64) if cond else (1, 64)),
            'w2': din(f'{hn}_w2', (64, odim)),
            'b2': din(f'{hn}_b2', (1, odim)),
        }
    oh_d = din('oh', (4, 8))
    maskrep_d = din('maskrep', (128, 4, 4, 8))
    outs_d = {n: nc.declare_dram_parameter(n, [8, o], f32, isOutput=True)
              for n, o in (('lane', 1), ('route', 1), ('tld', 1), ('tls', 2))}

    def APm(base, ap):
        return bass.AP(tensor=base.tensor, offset=base.offset, ap=ap)

    R = mybir.dt.float32r

    def mm(out, lhsT, rhs, r=True, **kw):
        if r:
            nc.tensor.matmul(out, lhsT.bitcast(R), rhs.bitcast(R), **kw)
        else:
            nc.tensor.matmul(out, lhsT.bitcast(mybir.dt.float32),
                             rhs.bitcast(mybir.dt.float32), **kw)

    lowp = nc.allow_low_precision("fp32r rounding of matmul inputs is intentional")
    lowp.__enter__()
    with tile.TileContext(nc) as tc:
        octx = ExitStack()
        with octx:
            consts = octx.enter_context(tc.tile_pool(name="consts", bufs=1))
            scr = octx.enter_context(tc.tile_pool(name="scr", bufs=3))
            s2outp = octx.enter_context(tc.tile_pool(name="s2out", bufs=4))

            def load(pool, shape, src, tag):
                t = pool.tile(list(shape), f32, tag=tag)
                nc.sync.dma_start(out=t[:], in_=src)
                return t

            c1b = load(consts, (128, 1), c1b_d[:], "c1b")
            s1b = load(consts, (128, 4), s1b_d[:], "s1b")
            s2b = load(consts, (128, 4), s2b_d[:], "s2b")
            s2bd = load(consts, (128, 1), s2bd_d[:], "s2bd")
            s3b = load(consts, (128, 4, 2), s3b_d[:], "s3b")
            s3bd = load(consts, (128, 2), s3bd_d[:], "s3bd")
            s4b = load(consts, (128, 4, 4), s4b_d[:], "s4b")
            s4bd = load(consts, (128, 4), s4bd_d[:], "s4bd")
            oh_t = load(consts, (4, 8), oh_d[:], "oh")
            maskrep = load(consts, (128, 4, 4, 8), maskrep_d[:], "maskrep")
            ones8 = consts.tile([1, 8], f32, tag="ones8")
            nc.vector.memset(ones8[:].bitcast(f32), 1.0)

            def zb(t, hp, wp, lead=1):
                """zero border rows/cols of [*, lead.., hp, wp] tile."""
                base = t[:]
                ps = list(base.ap[0])
                nc.gpsimd.memset(APm(base, [ps, [hp * wp, lead], [(hp - 1) * wp, 2], [1, wp]]).bitcast(f32), 0.0)
                nc.gpsimd.memset(APm(base, [ps, [hp * wp, lead], [wp, hp], [wp - 1, 2]]).bitcast(f32), 0.0)

            def evict(dst, src_ap, bias_ap, relu=True):
                nc.scalar.activation(dst, src_ap, AF.Relu if relu else AF.Identity,
                                     bias=bias_ap)

            s2out = []

            # ================= phase A: conv1 + maxpool + stage1 + stage2 ====
            actx = ExitStack()
            with actx:
                wA = actx.enter_context(tc.tile_pool(name="wA", bufs=1))
                c1p = actx.enter_context(tc.tile_pool(name="c1p", bufs=2))
                pbp = actx.enter_context(tc.tile_pool(name="pbp", bufs=4))
                hmp = actx.enter_context(tc.tile_pool(name="hmp", bufs=1))
                s1xp = actx.enter_context(tc.tile_pool(name="s1xp", bufs=2))
                s1yp = actx.enter_context(tc.tile_pool(name="s1yp", bufs=1))
                s1op = actx.enter_context(tc.tile_pool(name="s1op", bufs=1))
                s2yp = actx.enter_context(tc.tile_pool(name="s2yp", bufs=1))
                s2xp = actx.enter_context(tc.tile_pool(name="s2xp", bufs=2))
                s2scp = actx.enter_context(tc.tile_pool(name="s2scp", bufs=2))
                psA = actx.enter_context(tc.tile_pool(name="psA", bufs=6, space="PSUM"))

                c1w = load(wA, (96, 4, 128), c1w_d[:], "c1w")
                s1w = load(wA, (128, 4, 9, 128), s1w_d[:], "s1w")
                s2wc1 = load(wA, (128, 9, 128), s2wc1_d[:], "s2wc1")
                s2wr = load(wA, (128, 3, 9, 128), s2wr_d[:], "s2wr")
                s2wd = load(wA, (128, 128), s2wd_d[:], "s2wd")

                for pr in range(4):
                    # ---- conv1 + maxpool (paired) ----
                    hm = hmp.tile([128, 113, 56], f32, tag="hm")
                    nc.gpsimd.memset(hm[:, 0:1, :], 0.0)
                    x0 = s1xp.tile([128, 58, 58], f32, tag="s1x")
                    zb(x0, 58, 58)
                    for rc in range(7):
                        c1t = c1p.tile([96, 19, 112], f32, tag="c1t")
                        nc.sync.dma_start(out=c1t[0:48], in_=c1in_d[2 * pr, :, 16 * rc:16 * rc + 19, :])
                        nc.sync.dma_start(out=c1t[48:96], in_=c1in_d[2 * pr + 1, :, 16 * rc:16 * rc + 19, :])
                        for sub in range(4):
                            r0 = 16 * rc + 4 * sub
                            pt = psA.tile([128, 448], f32, tag="ps")
                            for a in range(4):
                                mm(pt[:], c1w[:, a, :],
                                                 c1t[:, a + 4 * sub:a + 4 * sub + 4, :],
                                                 start=(a == 0), stop=(a == 3))
                            pc = pbp.tile([128, 4, 113], f32, tag="pc")
                            nc.gpsimd.memset(pc[:, :, 0:1], 0.0)
                            evict(pc[:, :, 1:113], pt[:].rearrange("p (a b) -> p a b", a=4),
                                  c1b[:, 0:1])
                            nc.vector.tensor_max(hm[:, 1 + r0:5 + r0, :],
                                                 pc[:, :, 0:112:2], pc[:, :, 1:113:2])
                            nc.vector.tensor_max(hm[:, 1 + r0:5 + r0, :],
                                                 hm[:, 1 + r0:5 + r0, :], pc[:, :, 2:113:2])
                    nc.vector.tensor_max(x0[:, 1:57, 1:57],
                                         hm[:, 0:112:2, :], hm[:, 1:113:2, :])
                    nc.vector.tensor_max(x0[:, 1:57, 1:57],
                                         x0[:, 1:57, 1:57], hm[:, 2:113:2, :])

                    # ---- stage1 (paired) ----
                    def conv_s1(xin, cidx, out_tile, shortcut=None):
                        for ch in range(7):
                            r0 = ch * 8
                            pt = psA.tile([128, 448], f32, tag="ps")
                            for pos in range(9):
                                dy, dx = pos // 3, pos % 3
                                mm(pt[:], s1w[:, cidx, pos, :],
                                                 xin[:, r0 + dy:r0 + dy + 8, dx:dx + 56],
                                                 start=(pos == 0), stop=(pos == 8))
                            dst = out_tile[:, 1 + r0:1 + r0 + 8, 1:57]
                            ptv = pt[:].rearrange("p (a b) -> p a b", a=8)
                            if shortcut is None:
                                evict(dst, ptv, s1b[:, cidx:cidx + 1])
                            else:
                                tmp = scr.tile([128, 448], f32, tag="scr")
                                nc.vector.tensor_add(
                                    tmp[:].rearrange("p (a b) -> p a b", a=8), ptv,
                                    shortcut[:, 1 + r0:1 + r0 + 8, 1:57])
                                evict(dst, tmp[:].rearrange("p (a b) -> p a b", a=8),
                                      s1b[:, cidx:cidx + 1])

                    y1 = s1yp.tile([128, 58, 58], f32, tag="s1y")
                    zb(y1, 58, 58)
                    conv_s1(x0, 0, y1)
                    x1 = s1xp.tile([128, 58, 58], f32, tag="s1x")
                    zb(x1, 58, 58)
                    conv_s1(y1, 1, x1, shortcut=x0)
                    y2 = s1yp.tile([128, 58, 58], f32, tag="s1y")
                    zb(y2, 58, 58)
                    conv_s1(x1, 2, y2)
                    x2 = s1op.tile([128, 58, 58], f32, tag="s1o")
                    zb(x2, 58, 58)
                    conv_s1(y2, 3, x2, shortcut=x1)

                    # ---- stage2 (per image) ----
                    s2o = s2outp.tile([128, 2, 30, 30], f32, tag="s2o")
                    zb(s2o, 30, 30, lead=2)
                    for s in range(2):
                        xs1 = x2[64 * s:64 * s + 64]
                        y = s2yp.tile([128, 30, 30], f32, tag="s2y")
                        zb(y, 30, 30)
                        for ch in range(2):
                            r0 = ch * 14
                            pt = psA.tile([128, 392], f32, tag="ps")
                            for pos in range(9):
                                dy, dx = pos // 3, pos % 3
                                mm(
                                    pt[:], s2wc1[64 * s:64 * s + 64, pos, :],
                                    xs1[:, 2 * r0 + dy:2 * r0 + dy + 28:2, dx:dx + 56:2],
                                    start=(pos == 0), stop=(pos == 8))
                            evict(y[:, 1 + r0:1 + r0 + 14, 1:29],
                                  pt[:].rearrange("p (a b) -> p a b", a=14), s2b[:, 0:1])
                        sc = s2scp.tile([128, 784], f32, tag="s2sc")
                        for ch in range(2):
                            r0 = ch * 14
                            pt = psA.tile([128, 392], f32, tag="ps")
                            mm(pt[:], s2wd[64 * s:64 * s + 64, :],
                                             xs1[:, 1 + 2 * r0:1 + 2 * r0 + 28:2, 1:57:2],
                                             start=True, stop=True)
                            evict(sc[:, ch * 392:(ch + 1) * 392], pt[:], s2bd[:, 0:1],
                                  relu=False)

                        def conv_s2(xin, widx, dst_tile, bcol, shortcut=None, sc_flat=None):
                            for ch in range(2):
                                r0 = ch * 14
                                pt = psA.tile([128, 392], f32, tag="ps")
                                for pos in range(9):
                                    dy, dx = pos // 3, pos % 3
                                    mm(pt[:], s2wr[:, widx, pos, :],
                                                     xin[:, r0 + dy:r0 + dy + 14, dx:dx + 28],
                                                     start=(pos == 0), stop=(pos == 8))
                                dst = dst_tile[:, 1 + r0:1 + r0 + 14, 1:29]
                                ptv = pt[:].rearrange("p (a b) -> p a b", a=14)
                                if shortcut is None and sc_flat is None:
                                    evict(dst, ptv, s2b[:, bcol:bcol + 1])
                                else:
                                    tmp = scr.tile([128, 448], f32, tag="scr")
                                    tv = tmp[:, 0:392]
                                    if sc_flat is not None:
                                        nc.vector.tensor_add(tv, pt[:],
                                                             sc_flat[:, ch * 392:(ch + 1) * 392])
                                    else:
                                        nc.vector.tensor_add(
                                            tv.rearrange("p (a b) -> p a b", a=14), ptv,
                                            shortcut[:, 1 + r0:1 + r0 + 14, 1:29])
                                    evict(dst, tv.rearrange("p (a b) -> p a b", a=14),
                                          s2b[:, bcol:bcol + 1])

                        x3 = s2xp.tile([128, 30, 30], f32, tag="s2x")
                        zb(x3, 30, 30)
                        conv_s2(y, 0, x3, 1, sc_flat=sc)
                        y4 = s2yp.tile([128, 30, 30], f32, tag="s2y")
                        zb(y4, 30, 30)
                        conv_s2(x3, 1, y4, 2)
                        conv_s2(y4, 2, s2o[:, s], 3, shortcut=x3)
                    s2out.append(s2o)

            # ================= phase B: stage3 =================
            s3op = octx.enter_context(tc.tile_pool(name="s3op", bufs=1))
            s3o = s3op.tile([128, 8, 2, 16, 16], f32, tag="s3o")
            zb(s3o, 16, 16, lead=16)
            bctx = ExitStack()
            with bctx:
                wB = bctx.enter_context(tc.tile_pool(name="wB", bufs=1))
                s3tp = bctx.enter_context(tc.tile_pool(name="s3tp", bufs=2))
                psB = bctx.enter_context(tc.tile_pool(name="psB", bufs=6, space="PSUM"))

                s3wc1 = load(wB, (128, 9, 2, 128), s3wc1_d[:], "s3wc1")
                s3wr = [load(wB, (128, 2, 9, 2, 128), s3wr_d[cv], f"s3wr{cv}") for cv in range(3)]
                s3wd = load(wB, (128, 2, 128), s3wd_d[:], "s3wd")

                for pr in range(4):
                    xin = s2out[pr]
                    y = s3tp.tile([128, 2, 2, 16, 16], f32, tag="s3y")
                    zb(y, 16, 16, lead=4)
                    for cbo in range(2):
                        pt = psB.tile([128, 2, 196], f32, tag="ps")
                        for pos in range(9):
                            dy, dx = pos // 3, pos % 3
                            mm(pt[:], s3wc1[:, pos, cbo, :],
                                             xin[:, :, dy:dy + 28:2, dx:dx + 28:2],
                                             start=(pos == 0), stop=(pos == 8))
                        evict(y[:, :, cbo, 1:15, 1:15],
                              pt[:].rearrange("p i (a b) -> p i a b", a=14),
                              s3b[:, 0, cbo:cbo + 1])
                    sc = s3tp.tile([128, 2, 2, 196], f32, tag="s3sc")
                    for cbo in range(2):
                        pt = psB.tile([128, 2, 196], f32, tag="ps")
                        mm(pt[:], s3wd[:, cbo, :],
                                         xin[:, :, 1:29:2, 1:29:2], start=True, stop=True)
                        evict(sc[:, :, cbo], pt[:], s3bd[:, cbo:cbo + 1], relu=False)

                    def conv_s3(xin5, widx, dstf, bconv, shortcut=None, sc_t=None):
                        for cbo in range(2):
                            pt = psB.tile([128, 2, 196], f32, tag="ps")
                            k = 0
                            for cbi in range(2):
                                for pos in range(9):
                                    dy, dx = pos // 3, pos % 3
                                    mm(pt[:], s3wr[widx][:, cbi, pos, cbo, :],
                                                     xin5[:, :, cbi, dy:dy + 14, dx:dx + 14],
                                                     start=(k == 0), stop=(k == 17))
                                    k += 1
                            ptv = pt[:].rearrange("p i (a b) -> p i a b", a=14)
                            if sc_t is not None:
                                tmp = scr.tile([128, 448], f32, tag="scr")
                                nc.vector.tensor_add(tmp[:, 0:392], pt[:], sc_t[:, :, cbo])
                                src = tmp[:, 0:392].rearrange("p (i a b) -> p i a b", i=2, a=14)
                            elif shortcut is not None:
                                tmp = scr.tile([128, 448], f32, tag="scr")
                                nc.vector.tensor_add(
                                    tmp[:, 0:392].rearrange("p (i a b) -> p i a b", i=2, a=14),
                                    ptv, shortcut[:, :, cbo, 1:15, 1:15])
                                src = tmp[:, 0:392].rearrange("p (i a b) -> p i a b", i=2, a=14)
                            else:
                                src = ptv
                            evict(dstf(cbo), src, s3b[:, bconv, cbo:cbo + 1])

                    x5 = s3tp.tile([128, 2, 2, 16, 16], f32, tag="s3x")
                    zb(x5, 16, 16, lead=4)
                    conv_s3(y, 0, lambda cbo: x5[:, :, cbo, 1:15, 1:15], 1, sc_t=sc)
                    y6 = s3tp.tile([128, 2, 2, 16, 16], f32, tag="s3y")
                    zb(y6, 16, 16, lead=4)
                    conv_s3(x5, 1, lambda cbo: y6[:, :, cbo, 1:15, 1:15], 2)
                    conv_s3(y6, 2, lambda cbo: s3o[:, 2 * pr:2 * pr + 2, cbo, 1:15, 1:15],
                            3, shortcut=x5)

            # ================= phase C: stage4 + heads =================
            cctx = ExitStack()
            with cctx:
                wC = cctx.enter_context(tc.tile_pool(name="wC", bufs=3))
                wCd = cctx.enter_context(tc.tile_pool(name="wCd", bufs=1))
                s4tp = cctx.enter_context(tc.tile_pool(name="s4tp", bufs=1))
                s4yp = cctx.enter_context(tc.tile_pool(name="s4yp", bufs=2))
                headp = cctx.enter_context(tc.tile_pool(name="headp", bufs=1))
                psC = cctx.enter_context(tc.tile_pool(name="psC", bufs=6, space="PSUM"))

                def conv_s4_generic(wsrc_list, rhs_fn, n_cbi, n_pos, dstf, bias_fn):
                    pts = [psC.tile([128, 392], f32, tag="ps", name=f"ps4_{j}") for j in range(4)]
                    for cbi in range(n_cbi):
                        wt = load(wC, (128, 9, 4, 128) if n_pos == 9 else (128, 2, 4, 128),
                                  wsrc_list(cbi), "wc") if callable(wsrc_list) else wsrc_list[cbi]
                        for cbo in range(4):
                            for pos in range(n_pos):
                                mm(
                                    pts[cbo][:], wt[:, pos, cbo, :] if n_pos == 9
                                    else wt[:, cbi, cbo, :],
                                    rhs_fn(cbi, pos),
                                    start=(cbi == 0 and pos == 0),
                                    stop=(cbi == n_cbi - 1 and pos == n_pos - 1))
                    for cbo in range(4):
                        bias_fn(cbo, pts[cbo])

                # conv4_1 (s2)
                y = s4yp.tile([128, 8, 4, 9, 9], f32, tag="s4y")
                zb(y, 9, 9, lead=32)

                def rhs_41(cbi, pos):
                    dy, dx = pos // 3, pos % 3
                    return s3o[:, :, cbi, dy:dy + 14:2, dx:dx + 14:2].rearrange(
                        "p i a b -> p a b i")

                conv_s4_generic(lambda cbi: s4wc1_d[cbi], rhs_41, 2, 9,
                                None,
                                lambda cbo, pt: evict(
                                    y[:, :, cbo, 1:8, 1:8].rearrange("p i a b -> p a b i"),
                                    pt[:].rearrange("p (a b i) -> p a b i", a=7, b=7),
                                    s4b[:, 0, cbo:cbo + 1]))
                # downsample
                s4wd = load(wCd, (128, 2, 4, 128), s4wd_d[:], "s4wd")
                sc4 = s4tp.tile([128, 4, 392], f32, tag="s4sc")
                for cbo in range(4):
                    pt = psC.tile([128, 392], f32, tag="ps")
                    for cbi in range(2):
                        mm(pt[:], s4wd[:, cbi, cbo, :],
                           s3o[:, :, cbi, 1:15:2, 1:15:2].rearrange("p i a b -> p a b i"),
                           start=(cbi == 0), stop=(cbi == 1))
                    evict(sc4[:, cbo], pt[:], s4bd[:, cbo:cbo + 1], relu=False)

                def mk_conv_s4(widx, xin5, dstf, bconv, shortcut=None, sc_t=None):
                    def rhs(cbi, pos):
                        dy, dx = pos // 3, pos % 3
                        return xin5[:, :, cbi, dy:dy + 7, dx:dx + 7].rearrange(
                            "p i a b -> p a b i")

                    def bias(cbo, pt):
                        ptv = pt[:].rearrange("p i (a b) -> p i a b", a=7)
                        if sc_t is not None:
                            tmp = scr.tile([128, 448], f32, tag="scr")
                            nc.vector.tensor_add(tmp[:, 0:392], pt[:], sc_t[:, :, cbo])
                            src = tmp[:, 0:392].rearrange("p (i a b) -> p i a b", i=8, a=7)
                        elif shortcut is not None:
                            tmp = scr.tile([128, 448], f32, tag="scr")
                            nc.vector.tensor_add(
                                tmp[:, 0:392].rearrange("p (i a b) -> p i a b", i=8, a=7),
                                ptv, shortcut[:, :, cbo, 1:8, 1:8])
                            src = tmp[:, 0:392].rearrange("p (i a b) -> p i a b", i=8, a=7)
                        else:
                            src = ptv
                        evict(dstf(cbo), src, s4b[:, bconv, cbo:cbo + 1])

                    conv_s4_generic(lambda cbi: s4wr_d[widx, cbi], rhs, 4, 9, dstf, bias)

                x7 = s4tp.tile([128, 8, 4, 9, 9], f32, tag="s4x7")
                zb(x7, 9, 9, lead=32)
                mk_conv_s4(0, y, lambda cbo: x7[:, :, cbo, 1:8, 1:8].rearrange("p i a b -> p a b i"), 1, sc_t=sc4)
                y8 = s4yp.tile([128, 8, 4, 9, 9], f32, tag="s4y")
                zb(y8, 9, 9, lead=32)
                mk_conv_s4(1, x7, lambda cbo: y8[:, :, cbo, 1:8, 1:8].rearrange("p i a b -> p a b i"), 2)
                x9 = s4tp.tile([128, 8, 4, 9, 9], f32, tag="s4x9")
                zb(x9, 9, 9, lead=32)
                mk_conv_s4(2, y8, lambda cbo: x9[:, :, cbo, 1:8, 1:8].rearrange("p i a b -> p a b i"), 3, shortcut=x7)

                # ---- avgpool(sum) + heads ----
                Fcb = headp.tile([128, 4, 8], f32, tag="Fcb")
                fb = Fcb[:]
                nc.vector.reduce_sum(APm(fb, [list(fb.ap[0]), [1, 8], [8, 4]]),
                                     x9[:, :, :, 1:8, 1:8], axis=AX.XY)
                fm = headp.tile([128, 4, 4, 8], f32, tag="fm")
                for e in range(4):
                    nc.vector.tensor_mul(fm[:, e], Fcb[:], maskrep[:, e])

                hw = {}
                for hn in ('lane', 'route', 'tld', 'tls'):
                    cond = hn in ('lane', 'route')
                    odim = 2 if hn == 'tls' else 1
                    hw[hn] = {
                        'w1': load(headp, (128, 16, 64) if cond else (128, 4, 64),
                                   hd[hn]['w1'][:], f"hw1{hn}"),
                        'b1': load(headp, (4, 64) if cond else (1, 64), hd[hn]['b1'][:], f"hb1{hn}"),
                        'w2': load(headp, (64, odim), hd[hn]['w2'][:], f"hw2{hn}"),
                        'b2': load(headp, (1, odim), hd[hn]['b2'][:], f"hb2{hn}"),
                    }
                hctx = ExitStack()
                with hctx:
                    psH = hctx.enter_context(tc.tile_pool(name="psH", bufs=1, space="PSUM"))
                    for hn in ('lane', 'route', 'tld', 'tls'):
                        w = hw[hn]
                        cond = hn in ('lane', 'route')
                        odim = 2 if hn == 'tls' else 1
                        pt = psH.tile([64, 8], f32, tag="ph1", name=f"ph1{hn}")
                        if cond:
                            k = 0
                            for e in range(4):
                                for cb in range(4):
                                    mm(pt[:], w['w1'][:, 4 * e + cb, :],
                                       fm[:, e, cb, :], r=False, start=(k == 0), stop=False)
                                    k += 1
                            mm(pt[:], w['b1'][:], oh_t[:], r=False, start=False, stop=True)
                        else:
                            for cb in range(4):
                                mm(pt[:], w['w1'][:, cb, :], Fcb[:, cb, :],
                                   r=False, start=(cb == 0), stop=False)
                            mm(pt[:], w['b1'][:], ones8[:], r=False, start=False, stop=True)
                        h = headp.tile([64, 8], f32, tag=f"h{hn}")
                        nc.scalar.activation(h[:], pt[:], AF.Relu)
                        pt2 = psH.tile([odim, 8], f32, tag="ph2", name=f"ph2{hn}")
                        mm(pt2[:], w['w2'][:], h[:], r=False, start=True, stop=False)
                        mm(pt2[:], w['b2'][:], ones8[:], r=False, start=False, stop=True)
                        ot = headp.tile([odim, 8], f32, tag=f"o{hn}")
                        nc.scalar.activation(ot[:], pt2[:], AF.Copy)
                        od = outs_d[hn][:]
                        nc.sync.dma_start(out=APm(od, [[1, odim], [odim, 8]]), in_=ot[:])

    lowp.__exit__(None, None, None)
    nc.finalize()
    return nc


def _get_nc():
    if 'nc' not in _CACHE:
        _CACHE['nc'] = _build_bass()
    return _CACHE['nc']


def kernel(img, command, params):
    from concourse.bass_utils import run_bass_kernel_spmd

    img = np.asarray(img, np.float32)
    cmd = np.asarray(command)
    shared = _prep_shared(params)
    in_maps = []
    for c in range(N_CORES):
        sl = slice(c * B, (c + 1) * B)
        m = dict(shared)
        m.update(_prep_core(img[sl], cmd[sl]))
        in_maps.append(m)
    nc = _get_nc()
    res = run_bass_kernel_spmd(nc, in_maps, core_ids=list(range(N_CORES)))
    return tuple(np.concatenate([res.results[c][n] for c in range(N_CORES)], axis=0)
                 for n in ('lane', 'route', 'tld', 'tls'))
